# revision 29
# baseline (speedup 1.0000x reference)
"""Trainium2 Bass kernel for nn_DecoderBlock (B=8, C=1, S_TGT=2048, S_MEM=1024, D=512,
NH=2, DK=64, DHID=1024).

Strategy: data-parallel over batch B=8 across the 8 NeuronCores; all params
replicated. Per core, activations are kept in transposed layout [D, S] (d on
partitions) so every linear layer is a plain PE matmul with K=d contracted on
partitions. Host-side (free) work: input transpose + bf16 casts, packing the
two 64-dim heads into 128-wide operands, folding E1@D1 -> W1 and E2@D2 -> W2
(no nonlinearity between them), and transposing the output back.

Exact algebraic simplifications used:
 - bq/bk are additive constants along the softmax axes -> cancel exactly; skipped.
 - bv folded into the V eviction (softmax rows sum to 1 -> exact).
 - softmax computed without max-subtraction (inputs are O(1); identical in exact
   arithmetic, fp32-safe here).
 - LightAttention row-softmax A is folded as A = Eq * rq with rq absorbed into
   the Bm operand, so Z^T = (rq*r2*Bm_raw)^T @ Eq in two matmuls per head pair.

Fast path (the initializer's zero biases / identity LN affines, detected on
host): residual adds ride the PE as identity-matmul accumulations into PSUM,
evictions happen on the Scalar engine, and the residual stream is held in bf16
between layernorms. General path keeps fp32 residuals + DVE fused ops.
"""

import numpy as np
import ml_dtypes

B, C, S, SM, D, NH, DK, DHID = 8, 1, 2048, 1024, 512, 2, 64, 1024
DQ4 = float(DK) ** 0.25
NEG = -1.0e30
KC = D // 128  # 4 k-chunks
ST = S // 512  # 4 s-tiles of 512
NKV_SA = S // 128   # 16 kv chunks (self)
NKV_X = SM // 128   # 8 kv chunks (cross)

_BUILD_CACHE = {}
LAST_RESULT = None  # BassKernelResults of the most recent run (for profiling)


def _split_multiwait(nc):
    """This walrus build rejects >1 sem wait per instruction; hoist extra waits
    onto dedicated single-wait NoOps on the same engine (engines execute their
    streams in order, so wait-then-instruction is equivalent)."""
    import concourse.mybir as mybir
    n = 0
    for bbname, bassbb in list(nc.bb_map.items()):
        insts = bassbb.bb.instructions
        out = []
        changed = False
        for inst in insts:
            si = inst.sync_info
            if si is not None and si.on_wait and len(si.on_wait) > 1:
                waits = list(si.on_wait)
                for w in waits[:-1]:
                    nop = mybir.InstNoOp(name=f"splitw_{n}", ins=[], outs=[])
                    nop.engine = inst.engine
                    nop.sync_info = mybir.SyncInfo(on_wait=[w], on_update=[])
                    nc.register_instruction(nop)
                    out.append(nop)
                    n += 1
                si.on_wait = [waits[-1]]
                changed = True
            out.append(inst)
        if changed:
            insts[:] = out
    return n


def _build(variant):
    """Build the per-core Bass program.
    variant = (affine1, affine2, affine3, hasbo, hasbv)."""
    import concourse.bass as bass
    import concourse.tile as tile
    import concourse.mybir as mybir
    from contextlib import ExitStack

    aff = variant[:3]
    hasbo, hasbv = variant[3], variant[4]
    fast = not (hasbo or hasbv)

    dt = mybir.dt
    AF = mybir.ActivationFunctionType
    ALU = mybir.AluOpType

    nc = bass.Bass("TRN2", target_bir_lowering=False, debug=False, num_devices=8)

    f32, bf16 = dt.float32, dt.bfloat16
    din = {}
    din["ytb"] = nc.dram_tensor("ytb", [D, S], bf16, kind="ExternalInput")
    din["memtb"] = nc.dram_tensor("memtb", [D, SM], bf16, kind="ExternalInput")
    for p in ("sa", "x"):
        for w in ("wq", "wk", "wv"):
            din[f"{w}_{p}"] = nc.dram_tensor(f"{w}_{p}", [D, 128], bf16, kind="ExternalInput")
        din[f"wo_{p}"] = nc.dram_tensor(f"wo_{p}", [128, D], bf16, kind="ExternalInput")
        if hasbv:
            din[f"bv_{p}"] = nc.dram_tensor(f"bv_{p}", [128, 132], bf16, kind="ExternalInput")
        if hasbo:
            din[f"bo_{p}"] = nc.dram_tensor(f"bo_{p}", [128, KC], f32, kind="ExternalInput")
    din["w1"] = nc.dram_tensor("w1", [D, DHID], bf16, kind="ExternalInput")
    din["w2"] = nc.dram_tensor("w2", [DHID, D], bf16, kind="ExternalInput")
    din["maskq"] = nc.dram_tensor("maskq", [128, 64], f32, kind="ExternalInput")
    din["ident"] = nc.dram_tensor("ident", [128, 128], bf16, kind="ExternalInput")
    for i, need in enumerate(aff):
        if need:
            din[f"g{i+1}t"] = nc.dram_tensor(f"g{i+1}t", [D, S], f32, kind="ExternalInput")
            din[f"b{i+1}t"] = nc.dram_tensor(f"b{i+1}t", [D, S], f32, kind="ExternalInput")
    dout = nc.dram_tensor("out_t", [D, S], f32, kind="ExternalOutput")

    with tile.TileContext(nc) as tc, ExitStack() as ctx:
        ypool = ctx.enter_context(tc.tile_pool(name="ypool", bufs=4))
        bpool = ctx.enter_context(tc.tile_pool(name="bpool", bufs=4))
        wpool = ctx.enter_context(tc.tile_pool(name="wpool", bufs=1))
        apool = ctx.enter_context(tc.tile_pool(name="apool", bufs=1))
        hpool = ctx.enter_context(tc.tile_pool(name="hpool", bufs=2))
        spool = ctx.enter_context(tc.tile_pool(name="spool", bufs=8))
        psum = ctx.enter_context(tc.tile_pool(name="psum", bufs=6, space="PSUM"))
        psbm = ctx.enter_context(tc.tile_pool(name="psbm", bufs=2, space="PSUM"))

        # spread DMAs over the hardware DGE queues of several engines
        dma_engines = [nc.sync, nc.gpsimd]
        dma_rr = [0]

        def dma(dst, src):
            eng = dma_engines[dma_rr[0] % len(dma_engines)]
            dma_rr[0] += 1
            eng.dma_start(dst, src)

        zero1 = wpool.tile([128, 1], f32, name="zero1")
        nc.vector.memset(zero1[:], 0.0)
        eps1 = wpool.tile([128, 1], f32, name="eps1")
        nc.vector.memset(eps1[:], 1e-5)
        # scaled so the LN cross-partition matmul directly yields sums/512
        ones_c = wpool.tile([128, 1], f32, name="ones_c")
        nc.vector.memset(ones_c[:], 1.0 / 512.0)
        negone1 = wpool.tile([1, 1], f32, name="negone1")
        nc.vector.memset(negone1[:], -1.0)
        ones_r = wpool.tile([1, 128], f32, name="ones_r")
        nc.vector.memset(ones_r[:], 1.0)

        # ---- load inputs ----
        yTb = [bpool.tile([128, S], bf16, name=f"yTb{i}", tag="Yb") for i in range(KC)]
        memTb = [bpool.tile([128, SM], bf16, name=f"memTb{i}", tag="Mb") for i in range(KC)]
        wts = {}
        # self-attention K/V weights + yTb first: they gate the first matmuls
        for p in ("sa", "x"):
            for w in ("wk", "wv", "wq"):
                wts[f"{w}_{p}"] = wpool.tile([128, KC, 128], bf16, name=f"{w}_{p}_t")
            wts[f"wo_{p}"] = wpool.tile([128, D], bf16, name=f"wo_{p}_t")
        for w in ("wk", "wv"):
            dma(wts[f"{w}_sa"][:], din[f"{w}_sa"].ap().rearrange("(k p) e -> p k e", k=KC))
        for i in range(KC):
            dma(yTb[i][:], din["ytb"].ap()[128 * i:128 * (i + 1), :])
        dma(wts["wq_sa"][:], din["wq_sa"].ap().rearrange("(k p) e -> p k e", k=KC))
        dma(wts["wo_sa"][:], din["wo_sa"].ap())
        for i in range(KC):
            dma(memTb[i][:], din["memtb"].ap()[128 * i:128 * (i + 1), :])
        for w in ("wk", "wv", "wq"):
            dma(wts[f"{w}_x"][:], din[f"{w}_x"].ap().rearrange("(k p) e -> p k e", k=KC))
        dma(wts["wo_x"][:], din["wo_x"].ap())
        for p in ("sa", "x"):
            if hasbv:
                t = wpool.tile([128, 132], bf16, name=f"bv_{p}_t")
                dma(t[:], din[f"bv_{p}"].ap())
                wts[f"bv_{p}"] = t
            if hasbo:
                t = wpool.tile([128, KC], f32, name=f"bo_{p}_t")
                dma(t[:], din[f"bo_{p}"].ap())
                wts[f"bo_{p}"] = t
        w1t = wpool.tile([128, KC, DHID], bf16, name="w1t")
        dma(w1t[:], din["w1"].ap().rearrange("(k p) e -> p k e", k=KC))
        w2t = wpool.tile([128, DHID // 128, D], bf16, name="w2t")
        dma(w2t[:], din["w2"].ap().rearrange("(k p) e -> p k e", k=DHID // 128))
        maskq = wpool.tile([128, 64], f32, name="maskq_t")
        dma(maskq[:], din["maskq"].ap())
        ident = wpool.tile([128, 128], bf16, name="ident_t")
        dma(ident[:], din["ident"].ap())
        afft = {}
        for i, need in enumerate(aff):
            if need:
                g = [wpool.tile([128, S], f32, name=f"g{i+1}t_{m}") for m in range(KC)]
                b = [wpool.tile([128, S], f32, name=f"b{i+1}t_{m}") for m in range(KC)]
                for m in range(KC):
                    dma(g[m][:], din[f"g{i+1}t"].ap()[128 * m:128 * (m + 1), :])
                    dma(b[m][:], din[f"b{i+1}t"].ap()[128 * m:128 * (m + 1), :])
                afft[i] = (g, b)

        mm = nc.tensor.matmul

        def attn_kv(tag, kv, nkv):
            """K/V side of LightAttention: only depends on kv tiles.
            Returns the held Bm psum tile (raw Bm in [:, :128], Zk in col 128)."""
            wk, wv = wts[f"wk_{tag}"], wts[f"wv_{tag}"]
            Ft = apool.tile([128, NKV_SA * 128], bf16, name=f"Ft_{tag}", tag="Ft")
            Vt = apool.tile([128, NKV_SA, 132], bf16, name=f"Vt_{tag}", tag="Vt")
            nc.vector.memset(Vt[:, 0:nkv, 128:129], 1.0)
            for g in range(nkv // 4):
                pk = psum.tile([128, 512], f32, name=f"pk_{tag}_{g}", tag="mm512")
                for j in range(4):
                    sc = 4 * g + j
                    for kc in range(KC):
                        mm(pk[:, 128 * j:128 * (j + 1)],
                           kv[kc][:, 128 * sc:128 * (sc + 1)], wk[:, kc, :],
                           start=(kc == 0), stop=(kc == KC - 1))
                nc.scalar.activation(Ft[:, 512 * g:512 * (g + 1)], pk[:], AF.Exp,
                                     bias=zero1[:], scale=1.0 / DQ4)
                pv = psum.tile([128, 512], f32, name=f"pv_{tag}_{g}", tag="mm512")
                for j in range(4):
                    sc = 4 * g + j
                    for kc in range(KC):
                        mm(pv[:, 128 * j:128 * (j + 1)],
                           kv[kc][:, 128 * sc:128 * (sc + 1)], wv[:, kc, :],
                           start=(kc == 0), stop=(kc == KC - 1))
                if hasbv:
                    bv = wts[f"bv_{tag}"]
                    bva = bv[:, 0:128]
                    bv_bc = bass.AP(bva.tensor, bva.offset,
                                    [list(bva.ap[0]), [0, 4], list(bva.ap[1])])
                    nc.vector.scalar_tensor_tensor(
                        Vt[:, 4 * g:4 * g + 4, 0:128],
                        pv[:].rearrange("p (a b) -> p a b", a=4), 1.0,
                        bv_bc, ALU.mult, ALU.add)
                else:
                    nc.scalar.copy(Vt[:, 4 * g:4 * g + 4, 0:128],
                                   pv[:].rearrange("p (a b) -> p a b", a=4))
            pbm = psbm.tile([128, 132], f32, name=f"pbm_{tag}", tag="pbm")
            for sc in range(nkv):
                mm(pbm[:, 0:129], Ft[:, 128 * sc:128 * (sc + 1)], Vt[:, sc, 0:129],
                   start=(sc == 0), stop=(sc == nkv - 1))
            return pbm

        def attn_qo(tag, xq, pbm, use_mask, base_b, Xout):
            """Q side + output projection + residual (base_b: bf16 base tiles)."""
            wq, wo = wts[f"wq_{tag}"], wts[f"wo_{tag}"]
            Eq = apool.tile([128, S], bf16, name=f"Eq_{tag}", tag="Eq", bufs=2)
            qsum = spool.tile([128, ST], f32, name=f"qsum_{tag}")
            for st in range(ST):
                pq = psum.tile([128, 512], f32, name=f"pq_{tag}_{st}", tag="mm512")
                for kc in range(KC):
                    mm(pq[:], wq[:, kc, :], xq[kc][:, 512 * st:512 * (st + 1)],
                       start=(kc == 0), stop=(kc == KC - 1))
                if use_mask and st == 0:
                    nc.vector.tensor_add(pq[:, 0:64], pq[:, 0:64], maskq[:])
                nc.scalar.activation(Eq[:, 512 * st:512 * (st + 1)], pq[:], AF.Exp,
                                     bias=zero1[:], scale=1.0 / DQ4,
                                     accum_out=qsum[:, st:st + 1])

            # r_comb = 1/(zq*zk)
            zq = spool.tile([128, 1], f32, name=f"zq_{tag}")
            nc.vector.reduce_sum(zq[:], qsum[:], axis=mybir.AxisListType.X)
            zz = spool.tile([128, 1], f32, name=f"zz_{tag}")
            nc.vector.tensor_mul(zz[:], zq[:], pbm[:, 128:129])
            rc = spool.tile([128, 1], f32, name=f"rc_{tag}")
            nc.vector.reciprocal(rc[:], zz[:])

            # Bm block-diag evict with r_comb scaling
            Bm = apool.tile([128, 128], bf16, name=f"Bm_{tag}", tag="Bm")
            nc.vector.memset(Bm[:], 0.0)
            nc.scalar.activation(Bm[0:64, 0:64], pbm[0:64, 0:64], AF.Copy, scale=rc[0:64])
            nc.scalar.activation(Bm[64:128, 64:128], pbm[64:128, 64:128], AF.Copy,
                                 scale=rc[64:128])

            # Z^T (both heads in one matmul via block-diag Bm)
            ZcT = apool.tile([128, S], bf16, name=f"ZcT_{tag}", tag="ZcT")
            for st in range(ST):
                pz = psum.tile([128, 512], f32, name=f"pz_{tag}_{st}", tag="mm512")
                mm(pz[:], Bm[:], Eq[:, 512 * st:512 * (st + 1)], start=True, stop=True)
                nc.vector.tensor_copy(ZcT[:, 512 * st:512 * (st + 1)], pz[:])

            # O^T = Zc^T @ Wo; X = O (+bo) + base
            for mc in range(KC):
                for st in range(ST):
                    po = psum.tile([128, 512], f32, name=f"po_{tag}_{mc}_{st}", tag="mm512")
                    mm(po[:], wo[:, 128 * mc:128 * (mc + 1)],
                       ZcT[:, 512 * st:512 * (st + 1)], start=True, stop=not fast)
                    if fast:
                        # residual via PE identity-accumulate; evict on ACT
                        mm(po[:], ident[:], base_b[mc][:, 512 * st:512 * (st + 1)],
                           start=False, stop=True)
                        nc.scalar.copy(Xout[mc][:, 512 * st:512 * (st + 1)], po[:])
                    else:
                        nc.vector.scalar_tensor_tensor(
                            Xout[mc][:, 512 * st:512 * (st + 1)], po[:],
                            wts[f"bo_{tag}"][:, mc:mc + 1] if hasbo else 0.0,
                            base_b[mc][:, 512 * st:512 * (st + 1)],
                            ALU.add, ALU.add)

        def layernorm(X, iln, out_name, out_dt, out_tag, out_pool):
            """Global LN over all KC*[128,S] elements of X. Returns N tiles."""
            stats8 = spool.tile([128, 2 * KC], f32, name=f"st8_{out_name}")
            for mc in range(KC):
                st6 = spool.tile([128, ST, 6], f32, name=f"st6_{out_name}_{mc}", tag="st6")
                for j in range(ST):
                    nc.vector.bn_stats(st6[:, j, :], X[mc][:, 512 * j:512 * (j + 1)])
                mv = spool.tile([128, 2], f32, name=f"mv_{out_name}_{mc}", tag="mv")
                nc.vector.bn_aggr(mv[:], st6[:])
                nc.vector.tensor_copy(stats8[:, mc:mc + 1], mv[:, 0:1])
                # E[x^2] = var + mean^2
                nc.vector.scalar_tensor_tensor(stats8[:, KC + mc:KC + mc + 1],
                                               mv[:, 0:1], mv[:, 0:1], mv[:, 1:2],
                                               ALU.mult, ALU.add)
            # cross-partition sum via PE ones-matmul -> [1, 2*KC] on partition 0
            pr = psbm.tile([1, 2 * KC], f32, name=f"pr_{out_name}", tag="pbm")
            mm(pr[:], ones_c[:], stats8[:], start=True, stop=True)
            s8 = spool.tile([1, 2 * KC], f32, name=f"s8_{out_name}")
            nc.vector.tensor_copy(s8[:], pr[:])
            mu = spool.tile([1, 1], f32, name=f"mu_{out_name}")
            nc.vector.reduce_sum(mu[:], s8[:, 0:KC], axis=mybir.AxisListType.X)
            ex2 = spool.tile([1, 1], f32, name=f"ex2_{out_name}")
            nc.vector.reduce_sum(ex2[:], s8[:, KC:2 * KC], axis=mybir.AxisListType.X)
            var = spool.tile([1, 1], f32, name=f"var_{out_name}")
            nc.vector.scalar_tensor_tensor(var[:], mu[:], mu[:], ex2[:],
                                           ALU.mult, ALU.subtract)
            nc.vector.tensor_scalar_mul(var[:], var[:], -1.0)
            # rstd = exp(-0.5*ln(var+eps)) — stays in the exp/ln table set
            lnv = spool.tile([1, 1], f32, name=f"lnv_{out_name}")
            nc.scalar.activation(lnv[:], var[:], AF.Ln, bias=eps1[0:1])
            pair = spool.tile([1, 2], f32, name=f"pair_{out_name}")
            nc.scalar.activation(pair[:, 0:1], lnv[:], AF.Exp, bias=zero1[0:1], scale=-0.5)
            nc.vector.scalar_tensor_tensor(pair[:, 1:2], mu[:], pair[:, 0:1],
                                           negone1[:], ALU.mult, ALU.mult)
            # broadcast to all partitions via K=1 ones-matmul
            pb = psbm.tile([128, 2], f32, name=f"pb_{out_name}", tag="pbm")
            mm(pb[:], ones_r[:], pair[:], start=True, stop=True)
            bc = spool.tile([128, 2], f32, name=f"bc_{out_name}")
            nc.vector.tensor_copy(bc[:], pb[:])
            rstd_b, nmr_b = bc[:, 0:1], bc[:, 1:2]

            Nf = [out_pool.tile([128, S], out_dt, name=f"{out_name}{m}", tag=out_tag)
                  for m in range(KC)]
            has_aff = iln in afft
            for mc in range(KC):
                nc.vector.tensor_scalar(Nf[mc][:], X[mc][:], rstd_b, nmr_b,
                                        ALU.mult, ALU.add)
                if has_aff:
                    g, b = afft[iln]
                    nc.vector.tensor_mul(Nf[mc][:], Nf[mc][:], g[mc][:])
                    nc.vector.tensor_add(Nf[mc][:], Nf[mc][:], b[mc][:])
            return Nf

        bf16_dt = dt.bfloat16

        # ---- decoder block ----
        # self-attention (everything ready at start)
        pbm_sa = attn_kv("sa", yTb, NKV_SA)
        X1 = [bpool.tile([128, S], bf16_dt, name=f"x1_{m}", tag="Xb") for m in range(KC)]
        attn_qo("sa", yTb, pbm_sa, True, yTb, X1)

        # cross-attention K/V only needs mem — emit before LN1 so PE stays busy
        pbm_x = attn_kv("x", memTb, NKV_X)

        N1b = layernorm(X1, 0, "n1_", bf16_dt, "Nb", bpool)

        X2 = [bpool.tile([128, S], bf16_dt, name=f"x2_{m}", tag="Xb") for m in range(KC)]
        attn_qo("x", N1b, pbm_x, False, N1b, X2)

        N2b = layernorm(X2, 1, "n2_", bf16_dt, "Nb", bpool)

        # LFFN with folded weights: X3 = N2 + silu(N2b@W1)@W2
        X3 = [ypool.tile([128, S], f32, name=f"x3_{m}", tag="Yf") for m in range(KC)]
        for st in range(ST):
            Hb = hpool.tile([128, DHID // 128, 512], bf16_dt, name=f"Hb_{st}", tag="Hb")
            for hc in range(DHID // 128):
                ph = psum.tile([128, 512], f32, name=f"ph_{st}_{hc}", tag="mm512")
                for kc in range(KC):
                    mm(ph[:], w1t[:, kc, 128 * hc:128 * (hc + 1)],
                       N2b[kc][:, 512 * st:512 * (st + 1)],
                       start=(kc == 0), stop=(kc == KC - 1))
                nc.scalar.activation(Hb[:, hc, :], ph[:], AF.Silu, bias=zero1[:])
            for mc in range(KC):
                py = psum.tile([128, 512], f32, name=f"py_{st}_{mc}", tag="mm512")
                for hc in range(DHID // 128):
                    mm(py[:], w2t[:, hc, 128 * mc:128 * (mc + 1)], Hb[:, hc, :],
                       start=(hc == 0), stop=False)
                mm(py[:], ident[:], N2b[mc][:, 512 * st:512 * (st + 1)],
                   start=False, stop=True)
                if fast:
                    nc.scalar.copy(X3[mc][:, 512 * st:512 * (st + 1)], py[:])
                else:
                    nc.vector.tensor_copy(X3[mc][:, 512 * st:512 * (st + 1)], py[:])

        N3 = layernorm(X3, 2, "n3_", f32, "Nf3", ypool)
        for mc in range(KC):
            dma(dout.ap()[128 * mc:128 * (mc + 1), :], N3[mc][:])

    _split_multiwait(nc)
    return nc


def _host_pack(inputs):
    """Shard + pack inputs on the host. Returns (in_maps, variant)."""
    bf = ml_dtypes.bfloat16
    f32 = np.float32

    def cat_heads(w):  # (NH, D, DK) -> (D, NH*DK)
        return np.concatenate([w[h] for h in range(NH)], axis=1)

    y = np.asarray(inputs["y"], f32)      # (B, 1, S, D)
    mem = np.asarray(inputs["mem"], f32)  # (B, 1, SM, D)

    aff = tuple(
        not (np.all(np.asarray(inputs[g]) == 1.0) and np.all(np.asarray(inputs[b]) == 0.0))
        for g, b in (("g1", "b1"), ("g2", "b2"), ("g3", "b3")))
    hasbo = any(np.any(np.asarray(inputs[f"bo{t}"]) != 0.0) for t in ("_sa", "_x"))
    hasbv = any(np.any(np.asarray(inputs[f"bv{t}"]) != 0.0) for t in ("_sa", "_x"))
    variant = aff + (hasbo, hasbv)

    shared = {}
    for p, tag in (("sa", "_sa"), ("x", "_x")):
        shared[f"wq_{p}"] = np.ascontiguousarray(cat_heads(np.asarray(inputs[f"Wq{tag}"], f32))).astype(bf)
        shared[f"wk_{p}"] = np.ascontiguousarray(cat_heads(np.asarray(inputs[f"Wk{tag}"], f32))).astype(bf)
        shared[f"wv_{p}"] = np.ascontiguousarray(cat_heads(np.asarray(inputs[f"Wv{tag}"], f32))).astype(bf)
        shared[f"wo_{p}"] = np.ascontiguousarray(np.asarray(inputs[f"Wo{tag}"], f32)).astype(bf)
        if hasbv:
            bv2 = np.concatenate([np.asarray(inputs[f"bv{tag}"], f32)[h] for h in range(NH)])
            bvt = np.zeros((128, 132), f32)
            bvt[:, 0:128] = bv2[None, :]
            shared[f"bv_{p}"] = bvt.astype(bf)
        if hasbo:
            shared[f"bo_{p}"] = np.ascontiguousarray(
                np.asarray(inputs[f"bo{tag}"], f32).reshape(KC, 128).T).astype(f32)
    shared["w1"] = (np.asarray(inputs["E1"], np.float64) @ np.asarray(inputs["D1"], np.float64)).astype(f32).astype(bf)
    shared["w2"] = (np.asarray(inputs["E2"], np.float64) @ np.asarray(inputs["D2"], np.float64)).astype(f32).astype(bf)
    e_idx = np.arange(128) % 64
    s_idx = np.arange(64)
    shared["maskq"] = np.where(e_idx[:, None] > s_idx[None, :], NEG, 0.0).astype(f32)
    shared["ident"] = np.eye(128, dtype=f32).astype(bf)
    for i, need in enumerate(aff):
        if need:
            shared[f"g{i+1}t"] = np.ascontiguousarray(
                np.asarray(inputs[f"g{i+1}"], f32)[0].T).astype(f32)
            shared[f"b{i+1}t"] = np.ascontiguousarray(
                np.asarray(inputs[f"b{i+1}"], f32)[0].T).astype(f32)

    in_maps = []
    for b in range(B):
        m = dict(shared)
        ytr = np.ascontiguousarray(y[b, 0].T)           # (D, S)
        m["ytb"] = ytr.astype(bf)
        m["memtb"] = np.ascontiguousarray(mem[b, 0].T).astype(bf)
        in_maps.append(m)
    return in_maps, variant


def kernel(**inputs) -> np.ndarray:
    from concourse import bass_utils

    in_maps, variant = _host_pack(inputs)
    if variant not in _BUILD_CACHE:
        _BUILD_CACHE[variant] = _build(variant)
    nc = _BUILD_CACHE[variant]

    res = bass_utils.run_bass_kernel_spmd(nc, in_maps, core_ids=list(range(B)))
    global LAST_RESULT
    LAST_RESULT = res
    out = np.empty((B, C, S, D), np.float32)
    for b in range(B):
        out[b, 0] = res.results[b]["out_t"].T
    return out


# revision 37
# speedup vs baseline: 1.0360x; 1.0360x over previous
"""Trainium2 Bass kernel for nn_DecoderBlock (B=8, C=1, S_TGT=2048, S_MEM=1024, D=512,
NH=2, DK=64, DHID=1024).

Strategy: data-parallel over batch B=8 across the 8 NeuronCores; all params
replicated. Per core, activations are kept in transposed layout [D, S] (d on
partitions) so every linear layer is a plain PE matmul with K=d contracted on
partitions. Host-side (free) work: input transpose + bf16 casts, packing the
two 64-dim heads into 128-wide operands, folding E1@D1 -> W1 and E2@D2 -> W2
(no nonlinearity between them), and transposing the output back.

Exact algebraic simplifications used:
 - bq/bk are additive constants along the softmax axes -> cancel exactly; skipped.
 - bv folded into the V eviction (softmax rows sum to 1 -> exact).
 - softmax computed without max-subtraction (inputs are O(1); identical in exact
   arithmetic, fp32-safe here).
 - LightAttention row-softmax A is folded as A = Eq * rq with rq absorbed into
   the Bm operand, so Z^T = (rq*r2*Bm_raw)^T @ Eq in two matmuls per head pair.

Fast path (the initializer's zero biases / identity LN affines, detected on
host): residual adds ride the PE as identity-matmul accumulations into PSUM,
evictions happen on the Scalar engine, and the residual stream is held in bf16
between layernorms. General path keeps fp32 residuals + DVE fused ops.
"""

import numpy as np
import ml_dtypes

B, C, S, SM, D, NH, DK, DHID = 8, 1, 2048, 1024, 512, 2, 64, 1024
DQ4 = float(DK) ** 0.25
NEG = -1.0e30
KC = D // 128  # 4 k-chunks
ST = S // 512  # 4 s-tiles of 512
NKV_SA = S // 128   # 16 kv chunks (self)
NKV_X = SM // 128   # 8 kv chunks (cross)

_BUILD_CACHE = {}
LAST_RESULT = None  # BassKernelResults of the most recent run (for profiling)


def _split_multiwait(nc):
    """This walrus build rejects >1 sem wait per instruction; hoist extra waits
    onto dedicated single-wait NoOps on the same engine (engines execute their
    streams in order, so wait-then-instruction is equivalent)."""
    import concourse.mybir as mybir
    n = 0
    for bbname, bassbb in list(nc.bb_map.items()):
        insts = bassbb.bb.instructions
        out = []
        changed = False
        for inst in insts:
            si = inst.sync_info
            if si is not None and si.on_wait and len(si.on_wait) > 1:
                waits = list(si.on_wait)
                for w in waits[:-1]:
                    nop = mybir.InstNoOp(name=f"splitw_{n}", ins=[], outs=[])
                    nop.engine = inst.engine
                    nop.sync_info = mybir.SyncInfo(on_wait=[w], on_update=[])
                    nc.register_instruction(nop)
                    out.append(nop)
                    n += 1
                si.on_wait = [waits[-1]]
                changed = True
            out.append(inst)
        if changed:
            insts[:] = out
    return n


def _build(variant):
    """Build the per-core Bass program.
    variant = (affine1, affine2, affine3, hasbo, hasbv)."""
    import concourse.bass as bass
    import concourse.tile as tile
    import concourse.mybir as mybir
    from contextlib import ExitStack

    aff = variant[:3]
    hasbo, hasbv = variant[3], variant[4]
    fast = not (hasbo or hasbv)

    dt = mybir.dt
    AF = mybir.ActivationFunctionType
    ALU = mybir.AluOpType

    nc = bass.Bass("TRN2", target_bir_lowering=False, debug=False, num_devices=8)

    f32, bf16 = dt.float32, dt.bfloat16
    din = {}
    din["ytb"] = nc.dram_tensor("ytb", [D, S], bf16, kind="ExternalInput")
    din["memtb"] = nc.dram_tensor("memtb", [D, SM], bf16, kind="ExternalInput")
    for p in ("sa", "x"):
        for w in ("wq", "wk", "wv"):
            din[f"{w}_{p}"] = nc.dram_tensor(f"{w}_{p}", [D, 128], bf16, kind="ExternalInput")
        din[f"wo_{p}"] = nc.dram_tensor(f"wo_{p}", [128, D], bf16, kind="ExternalInput")
        if hasbv:
            din[f"bv_{p}"] = nc.dram_tensor(f"bv_{p}", [128, 132], bf16, kind="ExternalInput")
        if hasbo:
            din[f"bo_{p}"] = nc.dram_tensor(f"bo_{p}", [128, KC], f32, kind="ExternalInput")
    din["w1"] = nc.dram_tensor("w1", [D, DHID], bf16, kind="ExternalInput")
    din["w2"] = nc.dram_tensor("w2", [DHID, D], bf16, kind="ExternalInput")
    din["maskq"] = nc.dram_tensor("maskq", [128, 64], f32, kind="ExternalInput")
    din["wqsum_x"] = nc.dram_tensor("wqsum_x", [128, 1], f32, kind="ExternalInput")
    din["ident"] = nc.dram_tensor("ident", [128, 128], bf16, kind="ExternalInput")
    for i, need in enumerate(aff):
        if need:
            din[f"g{i+1}t"] = nc.dram_tensor(f"g{i+1}t", [D, S], f32, kind="ExternalInput")
            din[f"b{i+1}t"] = nc.dram_tensor(f"b{i+1}t", [D, S], f32, kind="ExternalInput")
    dout = nc.dram_tensor("out_t", [D, S], f32, kind="ExternalOutput")

    with tile.TileContext(nc) as tc, ExitStack() as ctx:
        ypool = ctx.enter_context(tc.tile_pool(name="ypool", bufs=4))
        bpool = ctx.enter_context(tc.tile_pool(name="bpool", bufs=4))
        wpool = ctx.enter_context(tc.tile_pool(name="wpool", bufs=1))
        apool = ctx.enter_context(tc.tile_pool(name="apool", bufs=1))
        hpool = ctx.enter_context(tc.tile_pool(name="hpool", bufs=2))
        spool = ctx.enter_context(tc.tile_pool(name="spool", bufs=8))
        psum = ctx.enter_context(tc.tile_pool(name="psum", bufs=6, space="PSUM"))
        psbm = ctx.enter_context(tc.tile_pool(name="psbm", bufs=2, space="PSUM"))

        # spread DMAs over the hardware DGE queues of several engines
        dma_engines = [nc.sync, nc.gpsimd]
        dma_rr = [0]

        def dma(dst, src):
            eng = dma_engines[dma_rr[0] % len(dma_engines)]
            dma_rr[0] += 1
            eng.dma_start(dst, src)

        zero1 = wpool.tile([128, 1], f32, name="zero1")
        nc.vector.memset(zero1[:], 0.0)
        eps1 = wpool.tile([128, 1], f32, name="eps1")
        nc.vector.memset(eps1[:], 1e-5)
        # scaled so the LN cross-partition matmul directly yields sums/512
        ones_c = wpool.tile([128, 1], f32, name="ones_c")
        nc.vector.memset(ones_c[:], 1.0 / 512.0)
        negone1 = wpool.tile([1, 1], f32, name="negone1")
        nc.vector.memset(negone1[:], -1.0)
        ones_r = wpool.tile([1, 128], f32, name="ones_r")
        nc.vector.memset(ones_r[:], 1.0)

        # ---- load inputs ----
        yTb = [bpool.tile([128, S], bf16, name=f"yTb{i}", tag="Yb") for i in range(KC)]
        memTb = [bpool.tile([128, SM], bf16, name=f"memTb{i}", tag="Mb") for i in range(KC)]
        wts = {}
        # self-attention K/V weights + yTb first: they gate the first matmuls
        for p in ("sa", "x"):
            for w in ("wk", "wv", "wq"):
                wts[f"{w}_{p}"] = wpool.tile([128, KC, 128], bf16, name=f"{w}_{p}_t")
            wts[f"wo_{p}"] = wpool.tile([128, D], bf16, name=f"wo_{p}_t")
        for w in ("wk", "wv", "wq"):
            dma(wts[f"{w}_sa"][:], din[f"{w}_sa"].ap().rearrange("(k p) e -> p k e", k=KC))
        for i in range(KC):
            dma(yTb[i][:], din["ytb"].ap()[128 * i:128 * (i + 1), :])
        dma(wts["wo_sa"][:], din["wo_sa"].ap())
        for i in range(KC):
            dma(memTb[i][:], din["memtb"].ap()[128 * i:128 * (i + 1), :])
        for w in ("wk", "wv", "wq"):
            dma(wts[f"{w}_x"][:], din[f"{w}_x"].ap().rearrange("(k p) e -> p k e", k=KC))
        dma(wts["wo_x"][:], din["wo_x"].ap())
        for p in ("sa", "x"):
            if hasbv:
                t = wpool.tile([128, 132], bf16, name=f"bv_{p}_t")
                dma(t[:], din[f"bv_{p}"].ap())
                wts[f"bv_{p}"] = t
            if hasbo:
                t = wpool.tile([128, KC], f32, name=f"bo_{p}_t")
                dma(t[:], din[f"bo_{p}"].ap())
                wts[f"bo_{p}"] = t
        w1t = wpool.tile([128, KC, DHID], bf16, name="w1t")
        dma(w1t[:], din["w1"].ap().rearrange("(k p) e -> p k e", k=KC))
        w2t = wpool.tile([128, DHID // 128, D], bf16, name="w2t")
        dma(w2t[:], din["w2"].ap().rearrange("(k p) e -> p k e", k=DHID // 128))
        maskq = wpool.tile([128, 64], f32, name="maskq_t")
        dma(maskq[:], din["maskq"].ap())
        wqsum_x = wpool.tile([128, 1], f32, name="wqsum_x_t")
        dma(wqsum_x[:], din["wqsum_x"].ap())
        ident = wpool.tile([128, 128], bf16, name="ident_t")
        dma(ident[:], din["ident"].ap())
        afft = {}
        for i, need in enumerate(aff):
            if need:
                g = [wpool.tile([128, S], f32, name=f"g{i+1}t_{m}") for m in range(KC)]
                b = [wpool.tile([128, S], f32, name=f"b{i+1}t_{m}") for m in range(KC)]
                for m in range(KC):
                    dma(g[m][:], din[f"g{i+1}t"].ap()[128 * m:128 * (m + 1), :])
                    dma(b[m][:], din[f"b{i+1}t"].ap()[128 * m:128 * (m + 1), :])
                afft[i] = (g, b)

        mm = nc.tensor.matmul

        def attn_kv(tag, kv, nkv):
            """K/V side of LightAttention: only depends on kv tiles.
            Returns the held Bm psum tile (raw Bm in [:, :128], Zk in col 128)."""
            wk, wv = wts[f"wk_{tag}"], wts[f"wv_{tag}"]
            Ft = apool.tile([128, NKV_SA * 128], bf16, name=f"Ft_{tag}", tag="Ft")
            Vt = apool.tile([128, NKV_SA, 132], bf16, name=f"Vt_{tag}", tag="Vt")
            nc.vector.memset(Vt[:, 0:nkv, 128:129], 1.0)
            for g in range(nkv // 4):
                pk = psum.tile([128, 512], f32, name=f"pk_{tag}_{g}", tag="mm512")
                for j in range(4):
                    sc = 4 * g + j
                    for kc in range(KC):
                        mm(pk[:, 128 * j:128 * (j + 1)],
                           kv[kc][:, 128 * sc:128 * (sc + 1)], wk[:, kc, :],
                           start=(kc == 0), stop=(kc == KC - 1))
                nc.scalar.activation(Ft[:, 512 * g:512 * (g + 1)], pk[:], AF.Exp,
                                     bias=zero1[:], scale=1.0 / DQ4)
                pv = psum.tile([128, 512], f32, name=f"pv_{tag}_{g}", tag="mm512")
                for j in range(4):
                    sc = 4 * g + j
                    for kc in range(KC):
                        mm(pv[:, 128 * j:128 * (j + 1)],
                           kv[kc][:, 128 * sc:128 * (sc + 1)], wv[:, kc, :],
                           start=(kc == 0), stop=(kc == KC - 1))
                if hasbv:
                    bv = wts[f"bv_{tag}"]
                    bva = bv[:, 0:128]
                    bv_bc = bass.AP(bva.tensor, bva.offset,
                                    [list(bva.ap[0]), [0, 4], list(bva.ap[1])])
                    nc.vector.scalar_tensor_tensor(
                        Vt[:, 4 * g:4 * g + 4, 0:128],
                        pv[:].rearrange("p (a b) -> p a b", a=4), 1.0,
                        bv_bc, ALU.mult, ALU.add)
                else:
                    nc.scalar.copy(Vt[:, 4 * g:4 * g + 4, 0:128],
                                   pv[:].rearrange("p (a b) -> p a b", a=4))
            pbm = psbm.tile([128, 132], f32, name=f"pbm_{tag}", tag="pbm")
            for sc in range(nkv):
                mm(pbm[:, 0:129], Ft[:, 128 * sc:128 * (sc + 1)], Vt[:, sc, 0:129],
                   start=(sc == 0), stop=(sc == nkv - 1))
            return pbm

        def attn_qo(tag, xq, pbm, use_mask, base_b, Xout, qraw=None,
                    qscale=None, qbias=None):
            """Q side + output projection + residual (base_b: bf16 base tiles).
            If qraw is given (pre-LN raw Q in SBUF), the Q matmuls are skipped and
            the LN is applied exactly inside the exp via per-partition scale/bias."""
            wq, wo = wts[f"wq_{tag}"], wts[f"wo_{tag}"]
            Eq = apool.tile([128, S], bf16, name=f"Eq_{tag}", tag="Eq", bufs=2)
            qsum = spool.tile([128, ST], f32, name=f"qsum_{tag}")
            for st in range(ST):
                if qraw is not None:
                    nc.scalar.activation(Eq[:, 512 * st:512 * (st + 1)],
                                         qraw[:, 512 * st:512 * (st + 1)], AF.Exp,
                                         bias=qbias[:], scale=qscale[:],
                                         accum_out=qsum[:, st:st + 1])
                    continue
                pq = psum.tile([128, 512], f32, name=f"pq_{tag}_{st}", tag="mm512")
                for kc in range(KC):
                    mm(pq[:], wq[:, kc, :], xq[kc][:, 512 * st:512 * (st + 1)],
                       start=(kc == 0), stop=(kc == KC - 1))
                if use_mask and st == 0:
                    nc.vector.tensor_add(pq[:, 0:64], pq[:, 0:64], maskq[:])
                nc.scalar.activation(Eq[:, 512 * st:512 * (st + 1)], pq[:], AF.Exp,
                                     bias=zero1[:], scale=1.0 / DQ4,
                                     accum_out=qsum[:, st:st + 1])

            # r_comb = 1/(zq*zk)
            zq = spool.tile([128, 1], f32, name=f"zq_{tag}")
            nc.vector.reduce_sum(zq[:], qsum[:], axis=mybir.AxisListType.X)
            zz = spool.tile([128, 1], f32, name=f"zz_{tag}")
            nc.vector.tensor_mul(zz[:], zq[:], pbm[:, 128:129])
            rc = spool.tile([128, 1], f32, name=f"rc_{tag}")
            nc.vector.reciprocal(rc[:], zz[:])

            # Bm block-diag evict with r_comb scaling
            Bm = apool.tile([128, 128], bf16, name=f"Bm_{tag}", tag="Bm")
            nc.vector.memset(Bm[:], 0.0)
            nc.scalar.activation(Bm[0:64, 0:64], pbm[0:64, 0:64], AF.Copy, scale=rc[0:64])
            nc.scalar.activation(Bm[64:128, 64:128], pbm[64:128, 64:128], AF.Copy,
                                 scale=rc[64:128])

            # Z^T (both heads in one matmul via block-diag Bm)
            ZcT = apool.tile([128, S], bf16, name=f"ZcT_{tag}", tag="ZcT")
            for st in range(ST):
                pz = psum.tile([128, 512], f32, name=f"pz_{tag}_{st}", tag="mm512")
                mm(pz[:], Bm[:], Eq[:, 512 * st:512 * (st + 1)], start=True, stop=True)
                nc.vector.tensor_copy(ZcT[:, 512 * st:512 * (st + 1)], pz[:])

            # O^T = Zc^T @ Wo; X = O (+bo) + base
            for mc in range(KC):
                for st in range(ST):
                    po = psum.tile([128, 512], f32, name=f"po_{tag}_{mc}_{st}", tag="mm512")
                    mm(po[:], wo[:, 128 * mc:128 * (mc + 1)],
                       ZcT[:, 512 * st:512 * (st + 1)], start=True, stop=not fast)
                    if fast:
                        # residual via PE identity-accumulate; evict on ACT
                        mm(po[:], ident[:], base_b[mc][:, 512 * st:512 * (st + 1)],
                           start=False, stop=True)
                        nc.scalar.copy(Xout[mc][:, 512 * st:512 * (st + 1)], po[:])
                    else:
                        nc.vector.scalar_tensor_tensor(
                            Xout[mc][:, 512 * st:512 * (st + 1)], po[:],
                            wts[f"bo_{tag}"][:, mc:mc + 1] if hasbo else 0.0,
                            base_b[mc][:, 512 * st:512 * (st + 1)],
                            ALU.add, ALU.add)

        def layernorm(X, iln, out_name, out_dt, out_tag, out_pool, out_dma=None):
            """Global LN over all KC*[128,S] elements of X. Returns N tiles."""
            stats8 = spool.tile([128, 2 * KC], f32, name=f"st8_{out_name}")
            for mc in range(KC):
                st6 = spool.tile([128, ST, 6], f32, name=f"st6_{out_name}_{mc}", tag="st6")
                for j in range(ST):
                    nc.vector.bn_stats(st6[:, j, :], X[mc][:, 512 * j:512 * (j + 1)])
                mv = spool.tile([128, 2], f32, name=f"mv_{out_name}_{mc}", tag="mv")
                nc.vector.bn_aggr(mv[:], st6[:])
                nc.vector.tensor_copy(stats8[:, mc:mc + 1], mv[:, 0:1])
                # E[x^2] = var + mean^2
                nc.vector.scalar_tensor_tensor(stats8[:, KC + mc:KC + mc + 1],
                                               mv[:, 0:1], mv[:, 0:1], mv[:, 1:2],
                                               ALU.mult, ALU.add)
            # cross-partition sum via PE ones-matmul -> [1, 2*KC] on partition 0
            pr = psbm.tile([1, 2 * KC], f32, name=f"pr_{out_name}", tag="pbm")
            mm(pr[:], ones_c[:], stats8[:], start=True, stop=True)
            s8 = spool.tile([1, 2 * KC], f32, name=f"s8_{out_name}")
            nc.vector.tensor_copy(s8[:], pr[:])
            mu = spool.tile([1, 1], f32, name=f"mu_{out_name}")
            nc.vector.reduce_sum(mu[:], s8[:, 0:KC], axis=mybir.AxisListType.X)
            ex2 = spool.tile([1, 1], f32, name=f"ex2_{out_name}")
            nc.vector.reduce_sum(ex2[:], s8[:, KC:2 * KC], axis=mybir.AxisListType.X)
            var = spool.tile([1, 1], f32, name=f"var_{out_name}")
            nc.vector.scalar_tensor_tensor(var[:], mu[:], mu[:], ex2[:],
                                           ALU.mult, ALU.subtract)
            nc.vector.tensor_scalar_mul(var[:], var[:], -1.0)
            # rstd = exp(-0.5*ln(var+eps)) — stays in the exp/ln table set
            lnv = spool.tile([1, 1], f32, name=f"lnv_{out_name}")
            nc.scalar.activation(lnv[:], var[:], AF.Ln, bias=eps1[0:1])
            pair = spool.tile([1, 2], f32, name=f"pair_{out_name}")
            nc.scalar.activation(pair[:, 0:1], lnv[:], AF.Exp, bias=zero1[0:1], scale=-0.5)
            nc.vector.scalar_tensor_tensor(pair[:, 1:2], mu[:], pair[:, 0:1],
                                           negone1[:], ALU.mult, ALU.mult)
            # broadcast to all partitions via K=1 ones-matmul
            pb = psbm.tile([128, 2], f32, name=f"pb_{out_name}", tag="pbm")
            mm(pb[:], ones_r[:], pair[:], start=True, stop=True)
            bc = spool.tile([128, 2], f32, name=f"bc_{out_name}")
            nc.vector.tensor_copy(bc[:], pb[:])
            rstd_b, nmr_b = bc[:, 0:1], bc[:, 1:2]

            Nf = [out_pool.tile([128, S], out_dt, name=f"{out_name}{m}", tag=out_tag)
                  for m in range(KC)]
            has_aff = iln in afft
            # st-major slices: downstream consumers of the first s-tile unblock
            # after 4 small ops. With an output DMA, go chunk-major instead so
            # whole chunks stream out as early as possible.
            if out_dma is None:
                for st in range(ST):
                    for mc in range(KC):
                        sl = slice(512 * st, 512 * (st + 1))
                        nc.vector.tensor_scalar(Nf[mc][:, sl], X[mc][:, sl],
                                                rstd_b, nmr_b, ALU.mult, ALU.add)
                        if has_aff:
                            g, b = afft[iln]
                            nc.vector.tensor_mul(Nf[mc][:, sl], Nf[mc][:, sl], g[mc][:, sl])
                            nc.vector.tensor_add(Nf[mc][:, sl], Nf[mc][:, sl], b[mc][:, sl])
            else:
                for mc in range(KC):
                    for st in range(ST):
                        sl = slice(512 * st, 512 * (st + 1))
                        nc.vector.tensor_scalar(Nf[mc][:, sl], X[mc][:, sl],
                                                rstd_b, nmr_b, ALU.mult, ALU.add)
                        if has_aff:
                            g, b = afft[iln]
                            nc.vector.tensor_mul(Nf[mc][:, sl], Nf[mc][:, sl], g[mc][:, sl])
                            nc.vector.tensor_add(Nf[mc][:, sl], Nf[mc][:, sl], b[mc][:, sl])
                        dma(out_dma.ap()[128 * mc:128 * (mc + 1), sl], Nf[mc][:, sl])
            return Nf, bc

        bf16_dt = dt.bfloat16

        # ---- decoder block ----
        # self-attention (everything ready at start)
        pbm_sa = attn_kv("sa", yTb, NKV_SA)
        X1 = [bpool.tile([128, S], bf16_dt, name=f"x1_{m}", tag="Xb") for m in range(KC)]
        attn_qo("sa", yTb, pbm_sa, True, yTb, X1)

        # cross-attention K/V only needs mem — emit before LN1 so PE stays busy
        pbm_x = attn_kv("x", memTb, NKV_X)

        N1b, bc1 = layernorm(X1, 0, "n1_", bf16_dt, "Nb", bpool)

        X2 = [bpool.tile([128, S], bf16_dt, name=f"x2_{m}", tag="Xb") for m in range(KC)]
        attn_qo("x", N1b, pbm_x, False, N1b, X2)

        N2b, _ = layernorm(X2, 1, "n2_", bf16_dt, "Nb", bpool)

        # LFFN with folded weights: X3 = N2 + silu(N2b@W1)@W2
        X3 = [ypool.tile([128, S], f32, name=f"x3_{m}", tag="Yf") for m in range(KC)]
        for st in range(ST):
            Hb = hpool.tile([128, DHID // 128, 512], bf16_dt, name=f"Hb_{st}", tag="Hb")
            for hc in range(DHID // 128):
                ph = psum.tile([128, 512], f32, name=f"ph_{st}_{hc}", tag="mm512")
                for kc in range(KC):
                    mm(ph[:], w1t[:, kc, 128 * hc:128 * (hc + 1)],
                       N2b[kc][:, 512 * st:512 * (st + 1)],
                       start=(kc == 0), stop=(kc == KC - 1))
                nc.scalar.activation(Hb[:, hc, :], ph[:], AF.Silu, bias=zero1[:])
            for mc in range(KC):
                py = psum.tile([128, 512], f32, name=f"py_{st}_{mc}", tag="mm512")
                for hc in range(DHID // 128):
                    mm(py[:], w2t[:, hc, 128 * mc:128 * (mc + 1)], Hb[:, hc, :],
                       start=(hc == 0), stop=False)
                mm(py[:], ident[:], N2b[mc][:, 512 * st:512 * (st + 1)],
                   start=False, stop=True)
                if fast:
                    nc.scalar.copy(X3[mc][:, 512 * st:512 * (st + 1)], py[:])
                else:
                    nc.vector.tensor_copy(X3[mc][:, 512 * st:512 * (st + 1)], py[:])

        layernorm(X3, 2, "n3_", f32, "Nf3", ypool, out_dma=dout)

    _split_multiwait(nc)
    return nc


def _host_pack(inputs):
    """Shard + pack inputs on the host. Returns (in_maps, variant)."""
    bf = ml_dtypes.bfloat16
    f32 = np.float32

    def cat_heads(w):  # (NH, D, DK) -> (D, NH*DK)
        return np.concatenate([w[h] for h in range(NH)], axis=1)

    y = np.asarray(inputs["y"], f32)      # (B, 1, S, D)
    mem = np.asarray(inputs["mem"], f32)  # (B, 1, SM, D)

    aff = tuple(
        not (np.all(np.asarray(inputs[g]) == 1.0) and np.all(np.asarray(inputs[b]) == 0.0))
        for g, b in (("g1", "b1"), ("g2", "b2"), ("g3", "b3")))
    hasbo = any(np.any(np.asarray(inputs[f"bo{t}"]) != 0.0) for t in ("_sa", "_x"))
    hasbv = any(np.any(np.asarray(inputs[f"bv{t}"]) != 0.0) for t in ("_sa", "_x"))
    variant = aff + (hasbo, hasbv)

    shared = {}
    for p, tag in (("sa", "_sa"), ("x", "_x")):
        shared[f"wq_{p}"] = np.ascontiguousarray(cat_heads(np.asarray(inputs[f"Wq{tag}"], f32))).astype(bf)
        shared[f"wk_{p}"] = np.ascontiguousarray(cat_heads(np.asarray(inputs[f"Wk{tag}"], f32))).astype(bf)
        shared[f"wv_{p}"] = np.ascontiguousarray(cat_heads(np.asarray(inputs[f"Wv{tag}"], f32))).astype(bf)
        shared[f"wo_{p}"] = np.ascontiguousarray(np.asarray(inputs[f"Wo{tag}"], f32)).astype(bf)
        if hasbv:
            bv2 = np.concatenate([np.asarray(inputs[f"bv{tag}"], f32)[h] for h in range(NH)])
            bvt = np.zeros((128, 132), f32)
            bvt[:, 0:128] = bv2[None, :]
            shared[f"bv_{p}"] = bvt.astype(bf)
        if hasbo:
            shared[f"bo_{p}"] = np.ascontiguousarray(
                np.asarray(inputs[f"bo{tag}"], f32).reshape(KC, 128).T).astype(f32)
    shared["w1"] = (np.asarray(inputs["E1"], np.float64) @ np.asarray(inputs["D1"], np.float64)).astype(f32).astype(bf)
    shared["w2"] = (np.asarray(inputs["E2"], np.float64) @ np.asarray(inputs["D2"], np.float64)).astype(f32).astype(bf)
    e_idx = np.arange(128) % 64
    s_idx = np.arange(64)
    shared["maskq"] = np.where(e_idx[:, None] > s_idx[None, :], NEG, 0.0).astype(f32)
    shared["ident"] = np.eye(128, dtype=f32).astype(bf)
    shared["wqsum_x"] = np.ascontiguousarray(
        cat_heads(np.asarray(inputs["Wq_x"], f32)).sum(axis=0)[:, None]).astype(f32)
    for i, need in enumerate(aff):
        if need:
            shared[f"g{i+1}t"] = np.ascontiguousarray(
                np.asarray(inputs[f"g{i+1}"], f32)[0].T).astype(f32)
            shared[f"b{i+1}t"] = np.ascontiguousarray(
                np.asarray(inputs[f"b{i+1}"], f32)[0].T).astype(f32)

    in_maps = []
    for b in range(B):
        m = dict(shared)
        ytr = np.ascontiguousarray(y[b, 0].T)           # (D, S)
        m["ytb"] = ytr.astype(bf)
        m["memtb"] = np.ascontiguousarray(mem[b, 0].T).astype(bf)
        in_maps.append(m)
    return in_maps, variant


def kernel(**inputs) -> np.ndarray:
    from concourse import bass_utils

    in_maps, variant = _host_pack(inputs)
    if variant not in _BUILD_CACHE:
        _BUILD_CACHE[variant] = _build(variant)
    nc = _BUILD_CACHE[variant]

    res = bass_utils.run_bass_kernel_spmd(nc, in_maps, core_ids=list(range(B)))
    global LAST_RESULT
    LAST_RESULT = res
    out = np.empty((B, C, S, D), np.float32)
    for b in range(B):
        out[b, 0] = res.results[b]["out_t"].T
    return out


# revision 39
# speedup vs baseline: 1.0500x; 1.0136x over previous
"""Trainium2 Bass kernel for nn_DecoderBlock (B=8, C=1, S_TGT=2048, S_MEM=1024, D=512,
NH=2, DK=64, DHID=1024).

Strategy: data-parallel over batch B=8 across the 8 NeuronCores; all params
replicated. Per core, activations are kept in transposed layout [D, S] (d on
partitions) so every linear layer is a plain PE matmul with K=d contracted on
partitions. Host-side (free) work: input transpose + bf16 casts, packing the
two 64-dim heads into 128-wide operands, folding E1@D1 -> W1 and E2@D2 -> W2
(no nonlinearity between them), and transposing the output back.

Exact algebraic simplifications used:
 - bq/bk are additive constants along the softmax axes -> cancel exactly; skipped.
 - bv folded into the V eviction (softmax rows sum to 1 -> exact).
 - softmax computed without max-subtraction (inputs are O(1); identical in exact
   arithmetic, fp32-safe here).
 - LightAttention row-softmax A is folded as A = Eq * rq with rq absorbed into
   the Bm operand, so Z^T = (rq*r2*Bm_raw)^T @ Eq in two matmuls per head pair.

Fast path (the initializer's zero biases / identity LN affines, detected on
host): residual adds ride the PE as identity-matmul accumulations into PSUM,
evictions happen on the Scalar engine, and the residual stream is held in bf16
between layernorms. General path keeps fp32 residuals + DVE fused ops.
"""

import numpy as np
import ml_dtypes

B, C, S, SM, D, NH, DK, DHID = 8, 1, 2048, 1024, 512, 2, 64, 1024
DQ4 = float(DK) ** 0.25
NEG = -1.0e30
KC = D // 128  # 4 k-chunks
ST = S // 512  # 4 s-tiles of 512
NKV_SA = S // 128   # 16 kv chunks (self)
NKV_X = SM // 128   # 8 kv chunks (cross)

_BUILD_CACHE = {}
LAST_RESULT = None  # BassKernelResults of the most recent run (for profiling)


def _split_multiwait(nc):
    """This walrus build rejects >1 sem wait per instruction; hoist extra waits
    onto dedicated single-wait NoOps on the same engine (engines execute their
    streams in order, so wait-then-instruction is equivalent)."""
    import concourse.mybir as mybir
    n = 0
    for bbname, bassbb in list(nc.bb_map.items()):
        insts = bassbb.bb.instructions
        out = []
        changed = False
        for inst in insts:
            si = inst.sync_info
            if si is not None and si.on_wait and len(si.on_wait) > 1:
                waits = list(si.on_wait)
                for w in waits[:-1]:
                    nop = mybir.InstNoOp(name=f"splitw_{n}", ins=[], outs=[])
                    nop.engine = inst.engine
                    nop.sync_info = mybir.SyncInfo(on_wait=[w], on_update=[])
                    nc.register_instruction(nop)
                    out.append(nop)
                    n += 1
                si.on_wait = [waits[-1]]
                changed = True
            out.append(inst)
        if changed:
            insts[:] = out
    return n


def _build(variant):
    """Build the per-core Bass program.
    variant = (affine1, affine2, affine3, hasbo, hasbv)."""
    import concourse.bass as bass
    import concourse.tile as tile
    import concourse.mybir as mybir
    from contextlib import ExitStack

    aff = variant[:3]
    hasbo, hasbv = variant[3], variant[4]
    fast = not (hasbo or hasbv)

    dt = mybir.dt
    AF = mybir.ActivationFunctionType
    ALU = mybir.AluOpType

    nc = bass.Bass("TRN2", target_bir_lowering=False, debug=False, num_devices=8)

    f32, bf16 = dt.float32, dt.bfloat16
    din = {}
    din["ytb"] = nc.dram_tensor("ytb", [D, S], bf16, kind="ExternalInput")
    din["memtb"] = nc.dram_tensor("memtb", [D, SM], bf16, kind="ExternalInput")
    for p in ("sa", "x"):
        for w in ("wq", "wk", "wv"):
            din[f"{w}_{p}"] = nc.dram_tensor(f"{w}_{p}", [D, 128], bf16, kind="ExternalInput")
        din[f"wo_{p}"] = nc.dram_tensor(f"wo_{p}", [128, D], bf16, kind="ExternalInput")
        if hasbv:
            din[f"bv_{p}"] = nc.dram_tensor(f"bv_{p}", [128, 132], bf16, kind="ExternalInput")
        if hasbo:
            din[f"bo_{p}"] = nc.dram_tensor(f"bo_{p}", [128, KC], f32, kind="ExternalInput")
    din["w1"] = nc.dram_tensor("w1", [D, DHID], bf16, kind="ExternalInput")
    din["w2"] = nc.dram_tensor("w2", [DHID, D], bf16, kind="ExternalInput")
    din["maskq"] = nc.dram_tensor("maskq", [128, 64], f32, kind="ExternalInput")
    din["wqsum_x"] = nc.dram_tensor("wqsum_x", [128, 1], f32, kind="ExternalInput")
    din["ident"] = nc.dram_tensor("ident", [128, 128], bf16, kind="ExternalInput")
    for i, need in enumerate(aff):
        if need:
            din[f"g{i+1}t"] = nc.dram_tensor(f"g{i+1}t", [D, S], f32, kind="ExternalInput")
            din[f"b{i+1}t"] = nc.dram_tensor(f"b{i+1}t", [D, S], f32, kind="ExternalInput")
    dout = nc.dram_tensor("out_t", [D, S], f32, kind="ExternalOutput")

    with tile.TileContext(nc) as tc, ExitStack() as ctx:
        ypool = ctx.enter_context(tc.tile_pool(name="ypool", bufs=4))
        bpool = ctx.enter_context(tc.tile_pool(name="bpool", bufs=4))
        wpool = ctx.enter_context(tc.tile_pool(name="wpool", bufs=1))
        apool = ctx.enter_context(tc.tile_pool(name="apool", bufs=1))
        hpool = ctx.enter_context(tc.tile_pool(name="hpool", bufs=2))
        spool = ctx.enter_context(tc.tile_pool(name="spool", bufs=8))
        psum = ctx.enter_context(tc.tile_pool(name="psum", bufs=6, space="PSUM"))
        psbm = ctx.enter_context(tc.tile_pool(name="psbm", bufs=2, space="PSUM"))

        # spread DMAs over the hardware DGE queues of several engines
        dma_engines = [nc.sync, nc.gpsimd]
        dma_rr = [0]

        def dma(dst, src):
            eng = dma_engines[dma_rr[0] % len(dma_engines)]
            dma_rr[0] += 1
            eng.dma_start(dst, src)

        zero1 = wpool.tile([128, 1], f32, name="zero1")
        nc.vector.memset(zero1[:], 0.0)
        eps1 = wpool.tile([128, 1], f32, name="eps1")
        nc.vector.memset(eps1[:], 1e-5)
        # scaled so the LN cross-partition matmul directly yields sums/512
        ones_c = wpool.tile([128, 1], f32, name="ones_c")
        nc.vector.memset(ones_c[:], 1.0 / 512.0)
        negone1 = wpool.tile([1, 1], f32, name="negone1")
        nc.vector.memset(negone1[:], -1.0)
        ones_r = wpool.tile([1, 128], f32, name="ones_r")
        nc.vector.memset(ones_r[:], 1.0)

        # ---- load inputs ----
        yTb = [bpool.tile([128, S], bf16, name=f"yTb{i}", tag="Yb") for i in range(KC)]
        memTb = [bpool.tile([128, SM], bf16, name=f"memTb{i}", tag="Mb") for i in range(KC)]
        wts = {}
        # self-attention K/V weights + yTb first: they gate the first matmuls
        for p in ("sa", "x"):
            for w in ("wk", "wv", "wq"):
                wts[f"{w}_{p}"] = wpool.tile([128, KC, 128], bf16, name=f"{w}_{p}_t")
            wts[f"wo_{p}"] = wpool.tile([128, D], bf16, name=f"wo_{p}_t")
        for w in ("wk", "wv", "wq"):
            dma(wts[f"{w}_sa"][:], din[f"{w}_sa"].ap().rearrange("(k p) e -> p k e", k=KC))
        # first s-tile of every chunk first: the first K/V/Q matmul groups only
        # need columns 0:512, so compute can start ~3x sooner
        for st in range(ST):
            for i in range(KC):
                sl = slice(512 * st, 512 * (st + 1))
                dma(yTb[i][:, sl], din["ytb"].ap()[128 * i:128 * (i + 1), sl])
        dma(wts["wo_sa"][:], din["wo_sa"].ap())
        for i in range(KC):
            dma(memTb[i][:], din["memtb"].ap()[128 * i:128 * (i + 1), :])
        for w in ("wk", "wv", "wq"):
            dma(wts[f"{w}_x"][:], din[f"{w}_x"].ap().rearrange("(k p) e -> p k e", k=KC))
        dma(wts["wo_x"][:], din["wo_x"].ap())
        for p in ("sa", "x"):
            if hasbv:
                t = wpool.tile([128, 132], bf16, name=f"bv_{p}_t")
                dma(t[:], din[f"bv_{p}"].ap())
                wts[f"bv_{p}"] = t
            if hasbo:
                t = wpool.tile([128, KC], f32, name=f"bo_{p}_t")
                dma(t[:], din[f"bo_{p}"].ap())
                wts[f"bo_{p}"] = t
        w1t = wpool.tile([128, KC, DHID], bf16, name="w1t")
        dma(w1t[:], din["w1"].ap().rearrange("(k p) e -> p k e", k=KC))
        w2t = wpool.tile([128, DHID // 128, D], bf16, name="w2t")
        dma(w2t[:], din["w2"].ap().rearrange("(k p) e -> p k e", k=DHID // 128))
        maskq = wpool.tile([128, 64], f32, name="maskq_t")
        dma(maskq[:], din["maskq"].ap())
        wqsum_x = wpool.tile([128, 1], f32, name="wqsum_x_t")
        dma(wqsum_x[:], din["wqsum_x"].ap())
        ident = wpool.tile([128, 128], bf16, name="ident_t")
        dma(ident[:], din["ident"].ap())
        afft = {}
        for i, need in enumerate(aff):
            if need:
                g = [wpool.tile([128, S], f32, name=f"g{i+1}t_{m}") for m in range(KC)]
                b = [wpool.tile([128, S], f32, name=f"b{i+1}t_{m}") for m in range(KC)]
                for m in range(KC):
                    dma(g[m][:], din[f"g{i+1}t"].ap()[128 * m:128 * (m + 1), :])
                    dma(b[m][:], din[f"b{i+1}t"].ap()[128 * m:128 * (m + 1), :])
                afft[i] = (g, b)

        mm = nc.tensor.matmul

        def attn_kv(tag, kv, nkv):
            """K/V side of LightAttention: only depends on kv tiles.
            Returns the held Bm psum tile (raw Bm in [:, :128], Zk in col 128)."""
            wk, wv = wts[f"wk_{tag}"], wts[f"wv_{tag}"]
            Ft = apool.tile([128, NKV_SA * 128], bf16, name=f"Ft_{tag}", tag="Ft")
            Vt = apool.tile([128, NKV_SA, 132], bf16, name=f"Vt_{tag}", tag="Vt")
            nc.vector.memset(Vt[:, 0:nkv, 128:129], 1.0)
            for g in range(nkv // 4):
                pk = psum.tile([128, 512], f32, name=f"pk_{tag}_{g}", tag="mm512")
                for j in range(4):
                    sc = 4 * g + j
                    for kc in range(KC):
                        mm(pk[:, 128 * j:128 * (j + 1)],
                           kv[kc][:, 128 * sc:128 * (sc + 1)], wk[:, kc, :],
                           start=(kc == 0), stop=(kc == KC - 1))
                nc.scalar.activation(Ft[:, 512 * g:512 * (g + 1)], pk[:], AF.Exp,
                                     bias=zero1[:], scale=1.0 / DQ4)
                pv = psum.tile([128, 512], f32, name=f"pv_{tag}_{g}", tag="mm512")
                for j in range(4):
                    sc = 4 * g + j
                    for kc in range(KC):
                        mm(pv[:, 128 * j:128 * (j + 1)],
                           kv[kc][:, 128 * sc:128 * (sc + 1)], wv[:, kc, :],
                           start=(kc == 0), stop=(kc == KC - 1))
                if hasbv:
                    bv = wts[f"bv_{tag}"]
                    bva = bv[:, 0:128]
                    bv_bc = bass.AP(bva.tensor, bva.offset,
                                    [list(bva.ap[0]), [0, 4], list(bva.ap[1])])
                    nc.vector.scalar_tensor_tensor(
                        Vt[:, 4 * g:4 * g + 4, 0:128],
                        pv[:].rearrange("p (a b) -> p a b", a=4), 1.0,
                        bv_bc, ALU.mult, ALU.add)
                else:
                    nc.scalar.copy(Vt[:, 4 * g:4 * g + 4, 0:128],
                                   pv[:].rearrange("p (a b) -> p a b", a=4))
            pbm = psbm.tile([128, 132], f32, name=f"pbm_{tag}", tag="pbm")
            for sc in range(nkv):
                mm(pbm[:, 0:129], Ft[:, 128 * sc:128 * (sc + 1)], Vt[:, sc, 0:129],
                   start=(sc == 0), stop=(sc == nkv - 1))
            return pbm

        def attn_qo(tag, xq, pbm, use_mask, base_b, Xout, qraw=None,
                    qscale=None, qbias=None):
            """Q side + output projection + residual (base_b: bf16 base tiles).
            If qraw is given (pre-LN raw Q in SBUF), the Q matmuls are skipped and
            the LN is applied exactly inside the exp via per-partition scale/bias."""
            wq, wo = wts[f"wq_{tag}"], wts[f"wo_{tag}"]
            Eq = apool.tile([128, S], bf16, name=f"Eq_{tag}", tag="Eq", bufs=2)
            qsum = spool.tile([128, ST], f32, name=f"qsum_{tag}")
            for st in range(ST):
                if qraw is not None:
                    nc.scalar.activation(Eq[:, 512 * st:512 * (st + 1)],
                                         qraw[:, 512 * st:512 * (st + 1)], AF.Exp,
                                         bias=qbias[:], scale=qscale[:],
                                         accum_out=qsum[:, st:st + 1])
                    continue
                pq = psum.tile([128, 512], f32, name=f"pq_{tag}_{st}", tag="mm512")
                for kc in range(KC):
                    mm(pq[:], wq[:, kc, :], xq[kc][:, 512 * st:512 * (st + 1)],
                       start=(kc == 0), stop=(kc == KC - 1))
                if use_mask and st == 0:
                    nc.vector.tensor_add(pq[:, 0:64], pq[:, 0:64], maskq[:])
                nc.scalar.activation(Eq[:, 512 * st:512 * (st + 1)], pq[:], AF.Exp,
                                     bias=zero1[:], scale=1.0 / DQ4,
                                     accum_out=qsum[:, st:st + 1])

            # r_comb = 1/(zq*zk)
            zq = spool.tile([128, 1], f32, name=f"zq_{tag}")
            nc.vector.reduce_sum(zq[:], qsum[:], axis=mybir.AxisListType.X)
            zz = spool.tile([128, 1], f32, name=f"zz_{tag}")
            nc.vector.tensor_mul(zz[:], zq[:], pbm[:, 128:129])
            rc = spool.tile([128, 1], f32, name=f"rc_{tag}")
            nc.vector.reciprocal(rc[:], zz[:])

            # Bm block-diag evict with r_comb scaling
            Bm = apool.tile([128, 128], bf16, name=f"Bm_{tag}", tag="Bm")
            nc.vector.memset(Bm[:], 0.0)
            nc.scalar.activation(Bm[0:64, 0:64], pbm[0:64, 0:64], AF.Copy, scale=rc[0:64])
            nc.scalar.activation(Bm[64:128, 64:128], pbm[64:128, 64:128], AF.Copy,
                                 scale=rc[64:128])

            # Z^T (both heads in one matmul via block-diag Bm)
            ZcT = apool.tile([128, S], bf16, name=f"ZcT_{tag}", tag="ZcT")
            for st in range(ST):
                pz = psum.tile([128, 512], f32, name=f"pz_{tag}_{st}", tag="mm512")
                mm(pz[:], Bm[:], Eq[:, 512 * st:512 * (st + 1)], start=True, stop=True)
                nc.vector.tensor_copy(ZcT[:, 512 * st:512 * (st + 1)], pz[:])

            # O^T = Zc^T @ Wo; X = O (+bo) + base
            for mc in range(KC):
                for st in range(ST):
                    po = psum.tile([128, 512], f32, name=f"po_{tag}_{mc}_{st}", tag="mm512")
                    mm(po[:], wo[:, 128 * mc:128 * (mc + 1)],
                       ZcT[:, 512 * st:512 * (st + 1)], start=True, stop=not fast)
                    if fast:
                        # residual via PE identity-accumulate; evict on ACT
                        mm(po[:], ident[:], base_b[mc][:, 512 * st:512 * (st + 1)],
                           start=False, stop=True)
                        nc.scalar.copy(Xout[mc][:, 512 * st:512 * (st + 1)], po[:])
                    else:
                        nc.vector.scalar_tensor_tensor(
                            Xout[mc][:, 512 * st:512 * (st + 1)], po[:],
                            wts[f"bo_{tag}"][:, mc:mc + 1] if hasbo else 0.0,
                            base_b[mc][:, 512 * st:512 * (st + 1)],
                            ALU.add, ALU.add)

        def layernorm(X, iln, out_name, out_dt, out_tag, out_pool, out_dma=None):
            """Global LN over all KC*[128,S] elements of X. Returns N tiles."""
            stats8 = spool.tile([128, 2 * KC], f32, name=f"st8_{out_name}")
            for mc in range(KC):
                st6 = spool.tile([128, ST, 6], f32, name=f"st6_{out_name}_{mc}", tag="st6")
                for j in range(ST):
                    nc.vector.bn_stats(st6[:, j, :], X[mc][:, 512 * j:512 * (j + 1)])
                mv = spool.tile([128, 2], f32, name=f"mv_{out_name}_{mc}", tag="mv")
                nc.vector.bn_aggr(mv[:], st6[:])
                nc.vector.tensor_copy(stats8[:, mc:mc + 1], mv[:, 0:1])
                # E[x^2] = var + mean^2
                nc.vector.scalar_tensor_tensor(stats8[:, KC + mc:KC + mc + 1],
                                               mv[:, 0:1], mv[:, 0:1], mv[:, 1:2],
                                               ALU.mult, ALU.add)
            # cross-partition sum via PE ones-matmul -> [1, 2*KC] on partition 0
            pr = psbm.tile([1, 2 * KC], f32, name=f"pr_{out_name}", tag="pbm")
            mm(pr[:], ones_c[:], stats8[:], start=True, stop=True)
            s8 = spool.tile([1, 2 * KC], f32, name=f"s8_{out_name}")
            nc.vector.tensor_copy(s8[:], pr[:])
            mu = spool.tile([1, 1], f32, name=f"mu_{out_name}")
            nc.vector.reduce_sum(mu[:], s8[:, 0:KC], axis=mybir.AxisListType.X)
            ex2 = spool.tile([1, 1], f32, name=f"ex2_{out_name}")
            nc.vector.reduce_sum(ex2[:], s8[:, KC:2 * KC], axis=mybir.AxisListType.X)
            var = spool.tile([1, 1], f32, name=f"var_{out_name}")
            nc.vector.scalar_tensor_tensor(var[:], mu[:], mu[:], ex2[:],
                                           ALU.mult, ALU.subtract)
            nc.vector.tensor_scalar_mul(var[:], var[:], -1.0)
            # rstd = exp(-0.5*ln(var+eps)) — stays in the exp/ln table set
            lnv = spool.tile([1, 1], f32, name=f"lnv_{out_name}")
            nc.scalar.activation(lnv[:], var[:], AF.Ln, bias=eps1[0:1])
            pair = spool.tile([1, 2], f32, name=f"pair_{out_name}")
            nc.scalar.activation(pair[:, 0:1], lnv[:], AF.Exp, bias=zero1[0:1], scale=-0.5)
            nc.vector.scalar_tensor_tensor(pair[:, 1:2], mu[:], pair[:, 0:1],
                                           negone1[:], ALU.mult, ALU.mult)
            # broadcast to all partitions via K=1 ones-matmul
            pb = psbm.tile([128, 2], f32, name=f"pb_{out_name}", tag="pbm")
            mm(pb[:], ones_r[:], pair[:], start=True, stop=True)
            bc = spool.tile([128, 2], f32, name=f"bc_{out_name}")
            nc.vector.tensor_copy(bc[:], pb[:])
            rstd_b, nmr_b = bc[:, 0:1], bc[:, 1:2]

            Nf = [out_pool.tile([128, S], out_dt, name=f"{out_name}{m}", tag=out_tag)
                  for m in range(KC)]
            has_aff = iln in afft
            # st-major slices: downstream consumers of the first s-tile unblock
            # after 4 small ops. With an output DMA, go chunk-major instead so
            # whole chunks stream out as early as possible.
            if out_dma is None:
                for st in range(ST):
                    for mc in range(KC):
                        sl = slice(512 * st, 512 * (st + 1))
                        nc.vector.tensor_scalar(Nf[mc][:, sl], X[mc][:, sl],
                                                rstd_b, nmr_b, ALU.mult, ALU.add)
                        if has_aff:
                            g, b = afft[iln]
                            nc.vector.tensor_mul(Nf[mc][:, sl], Nf[mc][:, sl], g[mc][:, sl])
                            nc.vector.tensor_add(Nf[mc][:, sl], Nf[mc][:, sl], b[mc][:, sl])
            else:
                for mc in range(KC):
                    for st in range(ST):
                        sl = slice(512 * st, 512 * (st + 1))
                        nc.vector.tensor_scalar(Nf[mc][:, sl], X[mc][:, sl],
                                                rstd_b, nmr_b, ALU.mult, ALU.add)
                        if has_aff:
                            g, b = afft[iln]
                            nc.vector.tensor_mul(Nf[mc][:, sl], Nf[mc][:, sl], g[mc][:, sl])
                            nc.vector.tensor_add(Nf[mc][:, sl], Nf[mc][:, sl], b[mc][:, sl])
                        dma(out_dma.ap()[128 * mc:128 * (mc + 1), sl], Nf[mc][:, sl])
            return Nf, bc

        bf16_dt = dt.bfloat16

        # ---- decoder block ----
        # self-attention (everything ready at start)
        pbm_sa = attn_kv("sa", yTb, NKV_SA)
        X1 = [bpool.tile([128, S], bf16_dt, name=f"x1_{m}", tag="Xb") for m in range(KC)]
        attn_qo("sa", yTb, pbm_sa, True, yTb, X1)

        # cross-attention K/V only needs mem — emit before LN1 so PE stays busy
        pbm_x = attn_kv("x", memTb, NKV_X)

        N1b, bc1 = layernorm(X1, 0, "n1_", bf16_dt, "Nb", bpool)

        X2 = [bpool.tile([128, S], bf16_dt, name=f"x2_{m}", tag="Xb") for m in range(KC)]
        attn_qo("x", N1b, pbm_x, False, N1b, X2)

        N2b, _ = layernorm(X2, 1, "n2_", bf16_dt, "Nb", bpool)

        # LFFN with folded weights: X3 = N2 + silu(N2b@W1)@W2
        X3 = [ypool.tile([128, S], f32, name=f"x3_{m}", tag="Yf") for m in range(KC)]
        for st in range(ST):
            Hb = hpool.tile([128, DHID // 128, 512], bf16_dt, name=f"Hb_{st}", tag="Hb")
            for hc in range(DHID // 128):
                ph = psum.tile([128, 512], f32, name=f"ph_{st}_{hc}", tag="mm512")
                for kc in range(KC):
                    mm(ph[:], w1t[:, kc, 128 * hc:128 * (hc + 1)],
                       N2b[kc][:, 512 * st:512 * (st + 1)],
                       start=(kc == 0), stop=(kc == KC - 1))
                nc.scalar.activation(Hb[:, hc, :], ph[:], AF.Silu, bias=zero1[:])
            for mc in range(KC):
                py = psum.tile([128, 512], f32, name=f"py_{st}_{mc}", tag="mm512")
                for hc in range(DHID // 128):
                    mm(py[:], w2t[:, hc, 128 * mc:128 * (mc + 1)], Hb[:, hc, :],
                       start=(hc == 0), stop=False)
                mm(py[:], ident[:], N2b[mc][:, 512 * st:512 * (st + 1)],
                   start=False, stop=True)
                if fast:
                    nc.scalar.copy(X3[mc][:, 512 * st:512 * (st + 1)], py[:])
                else:
                    nc.vector.tensor_copy(X3[mc][:, 512 * st:512 * (st + 1)], py[:])

        layernorm(X3, 2, "n3_", f32, "Nf3", ypool, out_dma=dout)

    _split_multiwait(nc)
    return nc


def _host_pack(inputs):
    """Shard + pack inputs on the host. Returns (in_maps, variant)."""
    bf = ml_dtypes.bfloat16
    f32 = np.float32

    def cat_heads(w):  # (NH, D, DK) -> (D, NH*DK)
        return np.concatenate([w[h] for h in range(NH)], axis=1)

    y = np.asarray(inputs["y"], f32)      # (B, 1, S, D)
    mem = np.asarray(inputs["mem"], f32)  # (B, 1, SM, D)

    aff = tuple(
        not (np.all(np.asarray(inputs[g]) == 1.0) and np.all(np.asarray(inputs[b]) == 0.0))
        for g, b in (("g1", "b1"), ("g2", "b2"), ("g3", "b3")))
    hasbo = any(np.any(np.asarray(inputs[f"bo{t}"]) != 0.0) for t in ("_sa", "_x"))
    hasbv = any(np.any(np.asarray(inputs[f"bv{t}"]) != 0.0) for t in ("_sa", "_x"))
    variant = aff + (hasbo, hasbv)

    shared = {}
    for p, tag in (("sa", "_sa"), ("x", "_x")):
        shared[f"wq_{p}"] = np.ascontiguousarray(cat_heads(np.asarray(inputs[f"Wq{tag}"], f32))).astype(bf)
        shared[f"wk_{p}"] = np.ascontiguousarray(cat_heads(np.asarray(inputs[f"Wk{tag}"], f32))).astype(bf)
        shared[f"wv_{p}"] = np.ascontiguousarray(cat_heads(np.asarray(inputs[f"Wv{tag}"], f32))).astype(bf)
        shared[f"wo_{p}"] = np.ascontiguousarray(np.asarray(inputs[f"Wo{tag}"], f32)).astype(bf)
        if hasbv:
            bv2 = np.concatenate([np.asarray(inputs[f"bv{tag}"], f32)[h] for h in range(NH)])
            bvt = np.zeros((128, 132), f32)
            bvt[:, 0:128] = bv2[None, :]
            shared[f"bv_{p}"] = bvt.astype(bf)
        if hasbo:
            shared[f"bo_{p}"] = np.ascontiguousarray(
                np.asarray(inputs[f"bo{tag}"], f32).reshape(KC, 128).T).astype(f32)
    shared["w1"] = (np.asarray(inputs["E1"], np.float64) @ np.asarray(inputs["D1"], np.float64)).astype(f32).astype(bf)
    shared["w2"] = (np.asarray(inputs["E2"], np.float64) @ np.asarray(inputs["D2"], np.float64)).astype(f32).astype(bf)
    e_idx = np.arange(128) % 64
    s_idx = np.arange(64)
    shared["maskq"] = np.where(e_idx[:, None] > s_idx[None, :], NEG, 0.0).astype(f32)
    shared["ident"] = np.eye(128, dtype=f32).astype(bf)
    shared["wqsum_x"] = np.ascontiguousarray(
        cat_heads(np.asarray(inputs["Wq_x"], f32)).sum(axis=0)[:, None]).astype(f32)
    for i, need in enumerate(aff):
        if need:
            shared[f"g{i+1}t"] = np.ascontiguousarray(
                np.asarray(inputs[f"g{i+1}"], f32)[0].T).astype(f32)
            shared[f"b{i+1}t"] = np.ascontiguousarray(
                np.asarray(inputs[f"b{i+1}"], f32)[0].T).astype(f32)

    in_maps = []
    for b in range(B):
        m = dict(shared)
        ytr = np.ascontiguousarray(y[b, 0].T)           # (D, S)
        m["ytb"] = ytr.astype(bf)
        m["memtb"] = np.ascontiguousarray(mem[b, 0].T).astype(bf)
        in_maps.append(m)
    return in_maps, variant


def kernel(**inputs) -> np.ndarray:
    from concourse import bass_utils

    in_maps, variant = _host_pack(inputs)
    if variant not in _BUILD_CACHE:
        _BUILD_CACHE[variant] = _build(variant)
    nc = _BUILD_CACHE[variant]

    res = bass_utils.run_bass_kernel_spmd(nc, in_maps, core_ids=list(range(B)))
    global LAST_RESULT
    LAST_RESULT = res
    out = np.empty((B, C, S, D), np.float32)
    for b in range(B):
        out[b, 0] = res.results[b]["out_t"].T
    return out
